# revision 1
# baseline (speedup 1.0000x reference)
"""Capsule dynamic-routing kernel for 8 Trainium2 NeuronCores.

Sharding: input-capsule dim IN_N=2048 split across the 8 cores (256 each),
full batch B=128 per core; per-round partial sums s [B,32,16] are AllReduced
(tiny). W is split 8x so each core touches only 4 MiB of weights.

Per round r in {1,2}:
  B. agreement: per j, o_j^T built by a base-0 PE transpose, then
     V_j[b,(i,e)] = sum_d o[b,j,d] W[j,i,d,e]  (PE, K=16)
     agr[b,j,i] = sum_e V_j * x  (DVE TT + reduce over e=8)
  C. softmax over j (ACT exp + DVE reduce/recip/mult), c in [b,(j,i)] layout
  D. s: y_j[b,(i,e)] = c[b,j,i] x[b,i,e] (DVE TT) -> PE-transpose to
     yT [(i,e), b] -> s^T_j[d, b] = sum_{(i,e)} wsd_j^T yT  (PE, K=128)
  E. s^T -> s [b,(j,d)] via PE transposes, AllReduce, squash.
"""

import os
from contextlib import ExitStack

import numpy as np

import concourse.bass as bass
import concourse.bacc as bacc
import concourse.tile as tile
from concourse import mybir, masks
from concourse.bass_utils import run_bass_kernel_spmd

B, IN_N, IN_D = 128, 2048, 8
CAPS, DIM = 32, 16
JD = CAPS * DIM          # 512
N_CORES = 8
I_LOC = IN_N // N_CORES  # 256
NQ = I_LOC // 16         # 16 chunks of 16 i's
IE = I_LOC * IN_D        # 2048 (i,e) per core
EPS = 1e-7
F32 = mybir.dt.float32

_CACHE = {}
LAST_RESULT = None


def _prep_core_inputs(x, W, k):
    sl = slice(k * I_LOC, (k + 1) * I_LOC)
    Wk = W[:, sl]                                   # [32, 256, 16, 8]
    xk = x[:, sl]                                   # [128, 256, 8]
    Wr = np.ascontiguousarray(Wk.transpose(1, 3, 0, 2)).reshape(I_LOC, IN_D, JD)
    xT = np.ascontiguousarray(xk.transpose(1, 2, 0))          # [i, e, b]

    # r0 dense layouts
    wr = Wr.reshape(NQ, 16 * IN_D, JD)
    wr = np.ascontiguousarray(wr.transpose(1, 0, 2)).reshape(128, NQ * JD)
    xt = xT.reshape(NQ, 16 * IN_D, B)
    xt = np.ascontiguousarray(xt.transpose(1, 0, 2)).reshape(128, NQ * B)

    # x in [b, (i,e)]
    xbie = np.ascontiguousarray(xk).reshape(B, IE)

    # wd3: V-mm moving. wd3[jg, d, m*IE + i*8+e] = W[4*jg+m, i, d, e]
    Wd = np.ascontiguousarray(Wk.transpose(0, 2, 1, 3)).reshape(CAPS, DIM, IE)
    wd3 = np.ascontiguousarray(
        Wd.reshape(16, 2, DIM, IE).transpose(0, 2, 1, 3)).reshape(16, DIM, 2 * IE)

    # wsd: s-mm stationary. wsd[(i_sub,e), j*256 + c*16 + d] = W[j, c*16+i_sub, d, e]
    # from Wr[i, e, jd=16j+d]: wsd[p=(i_sub,e), col] = Wr[c*16+i_sub, e, 16j+d]
    t = Wr.reshape(NQ, 16, IN_D, CAPS, DIM)         # [c, i_sub, e, j, d]
    wsd = np.ascontiguousarray(t.transpose(1, 2, 3, 0, 4)).reshape(128, CAPS * NQ * DIM)
    return {"wr": wr, "xt": xt, "xbie": xbie, "wdq": wd3, "wsd": wsd}


def _squash(nc, pool, s_ap, scale, obuf):
    sqt = pool.tile([128, JD], F32, tag="sq_t")
    nc.scalar.activation(sqt[:], s_ap, mybir.ActivationFunctionType.Square,
                         scale=float(scale))
    sq = pool.tile([128, CAPS], F32, tag="sq")
    nc.vector.tensor_reduce(sq[:], sqt[:].rearrange("p (j d) -> p j d", d=DIM),
                            axis=mybir.AxisListType.X, op=mybir.AluOpType.add)
    t1 = pool.tile([128, CAPS], F32, tag="sqa")
    nc.vector.tensor_scalar_add(t1[:], sq[:], 1.0)
    sqe = pool.tile([128, CAPS], F32, tag="sqf")
    nc.vector.tensor_scalar_add(sqe[:], sq[:], EPS)
    rt = pool.tile([128, CAPS], F32, tag="sqb")
    nc.scalar.activation(rt[:], sqe[:], mybir.ActivationFunctionType.Sqrt)
    den = pool.tile([128, CAPS], F32, tag="sqc")
    nc.vector.tensor_mul(den[:], t1[:], rt[:])
    rden = pool.tile([128, CAPS], F32, tag="sqd")
    nc.vector.reciprocal(rden[:], den[:])
    fac = pool.tile([128, CAPS], F32, tag="sqe")
    nc.vector.tensor_mul(fac[:], sq[:], rden[:])
    if scale != 1.0:
        nc.vector.tensor_scalar_mul(fac[:], fac[:], float(scale))
    nc.vector.tensor_mul(
        obuf[:].rearrange("p (j d) -> p j d", d=DIM),
        s_ap.rearrange("p (j d) -> p j d", d=DIM),
        fac[:].unsqueeze(-1).broadcast_to([128, CAPS, DIM]))


def _build(num_cores, reps=1):
    nc = bacc.Bacc("TRN2", target_bir_lowering=False, debug=False,
                   num_devices=num_cores)
    group = [list(range(num_cores))]

    wr_d = nc.dram_tensor("wr", [128, NQ * JD], F32, kind="ExternalInput")
    xt_d = nc.dram_tensor("xt", [128, NQ * B], F32, kind="ExternalInput")
    xbie_d = nc.dram_tensor("xbie", [128, IE], F32, kind="ExternalInput")
    wdq_d = nc.dram_tensor("wdq", [16, DIM, 2 * IE], F32, kind="ExternalInput")
    wsd_d = nc.dram_tensor("wsd", [128, CAPS * NQ * DIM], F32, kind="ExternalInput")
    out_d = nc.dram_tensor("out", [128, JD], F32, kind="ExternalOutput")

    with tile.TileContext(nc) as tc, ExitStack() as ctx:
        const = ctx.enter_context(tc.tile_pool(name="const", bufs=1))
        idp = ctx.enter_context(tc.tile_pool(name="idp", bufs=1))
        sm = ctx.enter_context(tc.tile_pool(name="small", bufs=2))
        big = ctx.enter_context(tc.tile_pool(name="big", bufs=1))
        wrp = ctx.enter_context(tc.tile_pool(name="wrp", bufs=3))
        wdp = ctx.enter_context(tc.tile_pool(name="wdp", bufs=2))
        sc = ctx.enter_context(tc.tile_pool(name="scratch", bufs=2))
        pp = ctx.enter_context(tc.tile_pool(name="pp", bufs=2, space="PSUM"))
        dram = ctx.enter_context(tc.tile_pool(name="dram", bufs=1, space="DRAM"))

        ident = idp.tile([128, 128], F32)
        masks.make_identity(nc, ident[:])

        for _rep in range(reps):
            xt_s = sc.tile([128, NQ * B], F32, tag="yj")
            nc.sync.dma_start(xt_s[:], xt_d.ap())
            xbie_s = const.tile([128, IE], F32, tag="xbie")
            nc.sync.dma_start(xbie_s[:], xbie_d.ap())
            wsd_s = const.tile([128, CAPS * NQ * DIM], F32, tag="wsd")
            nc.sync.dma_start(wsd_s[:], wsd_d.ap())

            blog = big.tile([128, CAPS * I_LOC], F32, tag="blog")  # [b, (j,i)]
            cexp = big.tile([128, CAPS * I_LOC], F32, tag="cexp")
            obuf = const.tile([128, JD], F32, tag="obuf")
            s_sb = const.tile([128, JD], F32, tag="s_sb")

            # ---------------- round 0: merged matmul ----------------
            ps0 = pp.tile([128, JD], F32, tag="pst")
            for c in range(NQ):
                wr_t = wrp.tile([128, JD], F32, tag="wr")
                nc.sync.dma_start(wr_t[:], wr_d.ap()[:, c * JD:(c + 1) * JD])
                nc.tensor.matmul(ps0[:], xt_s[:, c * B:(c + 1) * B], wr_t[:],
                                 start=(c == 0), stop=(c == NQ - 1))

            def allreduce_s(src_ap, scale, rnd):
                inb = dram.tile([128, JD], F32, tag=f"arin{rnd}")
                outb = dram.tile([128, JD], F32, tag=f"arout{rnd}",
                                 addr_space="Shared")
                nc.sync.dma_start(inb[:], src_ap)
                nc.gpsimd.collective_compute(
                    "AllReduce", mybir.AluOpType.add, replica_groups=group,
                    ins=[inb[:].opt()], outs=[outb[:].opt()])
                sf = sm.tile([128, JD], F32, tag="sfull")
                nc.sync.dma_start(sf[:], outb[:])
                _squash(nc, sm, sf[:], scale, obuf)

            s0s = sm.tile([128, JD], F32, tag="s0s")
            nc.scalar.copy(s0s[:], ps0[:])
            allreduce_s(s0s[:], 1.0 / CAPS, 0)

            # ---------------- rounds 1, 2 ----------------
            for rnd in (1, 2):
                # --- B: V + agreement ---
                for beta in range(16):
                    wd_t = wdp.tile([DIM, 2 * IE], F32, tag="wd")
                    nc.sync.dma_start(wd_t[:], wdq_d.ap()[beta])
                    for m in range(2):
                        j = 2 * beta + m
                        pt = pp.tile([128, 512], F32, tag="pst")
                        nc.tensor.transpose(
                            pt[:DIM, :128], obuf[:, DIM * j:DIM * (j + 1)],
                            ident[:])
                        oT_s = sm.tile([DIM, B], F32, tag="oT_s")
                        nc.scalar.copy(oT_s[:], pt[:DIM, :128])
                        for h in range(2):
                            pv = pp.tile([128, 1024], F32, tag="psv")
                            for n2 in range(2):
                                nc.tensor.matmul(
                                    pv[:, n2 * 512:(n2 + 1) * 512],
                                    oT_s[:],
                                    wd_t[:, m * IE + 1024 * h + 512 * n2:
                                         m * IE + 1024 * h + 512 * (n2 + 1)],
                                    start=True, stop=True)
                            prodv = sc.tile([128, 1024], F32, tag="yt")
                            nc.vector.tensor_mul(
                                prodv[:], pv[:],
                                xbie_s[:, 1024 * h:1024 * (h + 1)])
                            dst = blog[:, j * I_LOC + 128 * h:
                                       j * I_LOC + 128 * (h + 1)]
                            if rnd == 1:
                                nc.vector.tensor_reduce(
                                    dst,
                                    prodv[:].rearrange("p (i e) -> p i e", e=IN_D),
                                    axis=mybir.AxisListType.X,
                                    op=mybir.AluOpType.add)
                            else:
                                ag = sc.tile([128, 128], F32, tag="ag")
                                nc.vector.tensor_reduce(
                                    ag[:],
                                    prodv[:].rearrange("p (i e) -> p i e", e=IN_D),
                                    axis=mybir.AxisListType.X,
                                    op=mybir.AluOpType.add)
                                nc.vector.tensor_add(dst, dst, ag[:])

                # --- C: softmax over j ---
                nc.scalar.activation(cexp[:], blog[:],
                                     mybir.ActivationFunctionType.Exp)
                den = sm.tile([128, I_LOC], F32, tag="den")
                nc.vector.tensor_reduce(
                    den[:], cexp[:].rearrange("p (j i) -> p i j", j=CAPS),
                    axis=mybir.AxisListType.X, op=mybir.AluOpType.add)
                rden = sm.tile([128, I_LOC], F32, tag="rden")
                nc.vector.reciprocal(rden[:], den[:])
                nc.vector.tensor_mul(
                    cexp[:].rearrange("p (j i) -> p j i", j=CAPS),
                    cexp[:].rearrange("p (j i) -> p j i", j=CAPS),
                    rden[:].unsqueeze(1).broadcast_to([128, CAPS, I_LOC]))

                # --- D: y + s-mms ---
                sTa = pp.tile([128, JD], F32, tag="sTa", bufs=1, name=f"sTa_{rnd}")
                sTb = pp.tile([128, JD], F32, tag="sTb", bufs=1, name=f"sTb_{rnd}")
                nc.vector.memset(sTa[:], 0.0)
                nc.vector.memset(sTb[:], 0.0)
                for j in range(CAPS):
                    yj = sc.tile([128, IE], F32, tag="yj")
                    nc.vector.tensor_mul(
                        yj[:].rearrange("p (i e) -> p i e", e=IN_D),
                        xbie_s[:].rearrange("p (i e) -> p i e", e=IN_D),
                        cexp[:, j * I_LOC:(j + 1) * I_LOC]
                            .unsqueeze(-1).broadcast_to([128, I_LOC, IN_D]))
                    yt_s = sc.tile([128, IE], F32, tag="yt")
                    for cq in range(4):
                        pt = pp.tile([128, 512], F32, tag="pst")
                        for t in range(4):
                            c = 4 * cq + t
                            nc.tensor.transpose(
                                pt[:, 128 * t:128 * (t + 1)],
                                yj[:, 128 * c:128 * (c + 1)], ident[:])
                        nc.scalar.copy(
                            yt_s[:, 512 * cq:512 * (cq + 1)], pt[:])
                    grp, mj = j // 4, j % 4
                    sT_t = sTa if grp < 4 else sTb
                    cb = grp % 4
                    for c in range(NQ):
                        nc.tensor.matmul(
                            sT_t[32 * mj:32 * mj + DIM, 128 * cb:128 * (cb + 1)],
                            wsd_s[:, j * (NQ * DIM) + c * DIM:
                                  j * (NQ * DIM) + (c + 1) * DIM],
                            yt_s[:, 128 * c:128 * (c + 1)],
                            start=(c == 0), stop=(c == NQ - 1),
                            tile_position=(0, 32 * mj))

                # --- E: sT -> s_sb [b, (j,d)] ---
                for grp in range(8):
                    stt = sm.tile([128, 128], F32, tag="stt")
                    sT_t = sTa if grp < 4 else sTb
                    nc.scalar.copy(stt[:], sT_t[:, 128 * (grp % 4):128 * (grp % 4 + 1)])
                    ptb = pp.tile([128, 512], F32, tag="pst")
                    nc.tensor.transpose(ptb[:, :128], stt[:], ident[:])
                    src = ptb[:, :128].rearrange("p (m t) -> p m t", m=4)[:, :, :16]
                    nc.scalar.copy(
                        s_sb[:].rearrange("p (g m d) -> p g m d", g=8, d=16)[:, grp],
                        src)

                if rnd == 2:
                    allreduce_s(s_sb[:], 1.0, rnd)
                    nc.sync.dma_start(out_d.ap(), obuf[:])
                else:
                    allreduce_s(s_sb[:], 1.0, rnd)

    nc.compile()
    return nc


def kernel(x, W):
    global LAST_RESULT
    x = np.asarray(x, dtype=np.float32)
    W = np.asarray(W, dtype=np.float32)
    if "nc" not in _CACHE:
        _CACHE["nc"] = _build(N_CORES)
    nc = _CACHE["nc"]
    in_maps = [_prep_core_inputs(x, W, k) for k in range(N_CORES)]
    res = run_bass_kernel_spmd(nc, in_maps, list(range(N_CORES)),
                               trace=bool(os.environ.get("CAPS_TRACE")))
    LAST_RESULT = res
    out = res.results[0]["out"]
    return out.reshape(B, CAPS, DIM).astype(np.float32)

